# revision 24
# baseline (speedup 1.0000x reference)
"""Trainium2 Bass kernel for nn_AELossV2 (loss_fn).

Full inputs -> (pull, push) scalars.

Strategy: data-parallel over batch B=8 across 8 NeuronCores. Core k
processes mask[k] ([2048, 2048] u8, the only large tensor) plus its
[2048]-row slices of the small tensors, producing 5 scalar partials:
  [pull_num, pull_den, diag_masked_cnt, mask_cnt_raw, abssum]
The host sums partials over cores and forms the two output scalars.

Per-core math (s = sigmoid(avg_row), thr = 0.6):
  abssum = sum_ij mask[i,j] * |s[j] - s[i]|     (dist_mask is implicit:
           pairs excluded by dist_mask have s equal in EVERY batch row,
           so their |s[j]-s[i]| term is 0 in every core's sum already)
  count  = sum_ij mask[i,j] - sum_i mask[i,i]   (- host-side correction
           for duplicate s-columns, which never occur for random data)
  push   = (thr*count - abssum) / count
  pull   = sum(tag * c) / sum(c),  tag = softplus(x) - x * (g > 0)

Engine split per [128, 2048] tile of the [N, N] plane:
  ACT:  at = Abs(s_brd_psum - s_col)   (|d|; reads s_brd from PSUM)
  DVE:  P  = min(mask_u8, at)          (masking without a cast: u8 1
        converts to 1.0 > |d|, so min selects |d|; 0 selects 0)
  PE :  column sums of P accumulated into PSUM  (-> abssum)
  DMA:  software-DGE accum-DMAs fold the mask tiles into u8
        accumulators (values <= 16) -> count costs ~no engine time.
s_brd is built by a PE rank-1 broadcast (block-identity @ s_row) and
stays in PSUM; s appears in both layouts via two ACT Sigmoid calls on
two differently-laid-out DMA copies of avg, so no transpose is needed.
"""

import sys
from contextlib import ExitStack

import numpy as np

try:
    import concourse.bass  # noqa: F401
except ImportError:  # pragma: no cover
    sys.path.insert(0, "/opt/trn_rl_repo")

B = 8
N = 2048
P = 128
NT = N // P  # 16 tiles per plane
THR = 0.5 + 0.1
N_CORES = 8
N_PARTIALS = 8  # padded partials vector


def build_kernel():
    import concourse.bass as bass
    import concourse.tile as tile
    from concourse import bacc, mybir

    f16 = mybir.dt.float16
    f32 = mybir.dt.float32
    u8 = mybir.dt.uint8
    AF = mybir.ActivationFunctionType
    OP = mybir.AluOpType
    AX = mybir.AxisListType

    nc = bacc.Bacc("TRN2", target_bir_lowering=False, debug=False)

    mask_d = nc.dram_tensor("mask", [N, N], u8, kind="ExternalInput")
    avg_d = nc.dram_tensor("avg", [NT, P], f32, kind="ExternalInput")
    x_d = nc.dram_tensor("x", [NT, P], f32, kind="ExternalInput")
    g_d = nc.dram_tensor("g", [NT, P], f32, kind="ExternalInput")
    c_d = nc.dram_tensor("cen", [NT, P], f32, kind="ExternalInput")
    out_d = nc.dram_tensor("out", [N_PARTIALS, 1], f32, kind="ExternalOutput")

    part_d = nc.dram_tensor("part_scratch", [P, N_PARTIALS], f32)

    with tile.TileContext(nc) as tc, ExitStack() as ctx:
        const = ctx.enter_context(tc.tile_pool(name="const", bufs=1))
        mpool = ctx.enter_context(tc.tile_pool(name="masku8", bufs=1))
        apool = ctx.enter_context(tc.tile_pool(name="absd", bufs=4))
        ppool = ctx.enter_context(tc.tile_pool(name="prod", bufs=4))
        pspool = ctx.enter_context(
            tc.tile_pool(name="ps", bufs=1, space=bass.MemorySpace.PSUM)
        )

        # ---- DMA plan: two tiny avg loads first, then the mask tiles
        # split across the two HWDGE queues (sync + scalar). Small pull
        # inputs ride the software DGE so they never queue behind masks.
        avg_pt = const.tile([P, NT], f32)
        nc.sync.dma_start(avg_pt[:], avg_d.ap().rearrange("t p -> p t"))
        avg_row = const.tile([NT, P], f32)
        nc.sync.dma_start(avg_row[:], avg_d.ap())
        mts = []
        for t in range(NT):
            mt = mpool.tile([P, N], u8, tag=f"mt{t}")
            eng = nc.sync if t % 2 == 0 else nc.scalar
            eng.dma_start(mt[:], mask_d.ap()[t * P : (t + 1) * P, :])
            mts.append(mt)
        x_sb = const.tile([NT, P], f32)
        g_sb = const.tile([NT, P], f32)
        c_sb = const.tile([NT, P], f32)
        nc.gpsimd.dma_start(x_sb[:], x_d.ap())
        nc.gpsimd.dma_start(g_sb[:], g_d.ap())
        nc.gpsimd.dma_start(c_sb[:], c_d.ap())

        # ---- s in both layouts via two Sigmoid calls (table set 2 also
        # holds Abs and Copy, so the whole plane needs one table load)
        sc_f32 = const.tile([P, NT], f32)
        nc.scalar.activation(sc_f32[:], avg_pt[:], AF.Sigmoid)
        sr_f32 = const.tile([NT, P], f32)
        nc.scalar.activation(sr_f32[:], avg_row[:], AF.Sigmoid)
        s16c = const.tile([P, NT], f16)
        nc.vector.tensor_copy(s16c[:], sc_f32[:])
        sneg_col = const.tile([P, NT], f32)
        nc.vector.tensor_scalar(
            out=sneg_col[:], in0=s16c[:], scalar1=-1.0, scalar2=None, op0=OP.mult
        )
        s16row = const.tile([NT, P], f16)
        nc.vector.tensor_copy(s16row[:], sr_f32[:])

        # ---- s_brd: PE rank-1 broadcast into PSUM (stays there; the
        # per-tile Abs reads PSUM directly). lhsT is a block-identity
        # selector column broadcast along the free axis.
        iota_t = const.tile([NT, NT], mybir.dt.int16)
        nc.gpsimd.iota(iota_t[:], pattern=[[1, NT]], channel_multiplier=-1)
        id16 = const.tile([NT, NT], f16)
        nc.vector.tensor_scalar(
            out=id16[:], in0=iota_t[:], scalar1=0.0, scalar2=None, op0=OP.is_equal
        )
        psum_brd = pspool.tile([P, N], f32)
        for t in range(NT):
            nc.tensor.matmul(
                psum_brd[:, t * P : (t + 1) * P],
                id16[:, t : t + 1].broadcast_to([NT, P]),
                s16row[:, :],
                start=True,
                stop=True,
            )

        partials = const.tile([P, N_PARTIALS], f32)
        nc.vector.memset(partials[:], 0.0)
        ones = const.tile([P, 1], f16)
        nc.vector.memset(ones[:], 1.0)

        # pull pieces that need no ACT (its tables are busy with set 2)
        tgt = const.tile([NT, P], f32)
        nc.vector.tensor_scalar(
            out=tgt[:], in0=g_sb[:], scalar1=0.0, scalar2=None, op0=OP.is_gt
        )
        xt = const.tile([NT, P], f32)
        nc.vector.tensor_tensor(out=xt[:], in0=x_sb[:], in1=tgt[:], op=OP.mult)
        nc.vector.tensor_reduce(
            out=partials[0:NT, 1:2], in_=c_sb[:], axis=AX.X, op=OP.add
        )

        # ---------------- the [N, N] plane loop ----------------
        psum_abs = pspool.tile([1, N], f32)
        for t in range(NT):
            mt = mts[t]
            # |s_j - s_i| on ACT: Abs(1.0 * s_brd + (-s_col)), PSUM src
            at = apool.tile([P, N], f16)
            nc.scalar.activation(
                at[:], psum_brd[:, :], AF.Abs, bias=sneg_col[:, t : t + 1]
            )
            # masked |d| on DVE: mask==1 -> 1.0 > |d|, min picks |d|
            pt_ = ppool.tile([P, N], f16)
            nc.vector.tensor_tensor(out=pt_[:], in0=mt[:], in1=at[:], op=OP.min)
            # column sums into the PSUM accumulator
            for c4 in range(4):
                nc.tensor.matmul(
                    psum_abs[0:1, c4 * 512 : (c4 + 1) * 512],
                    ones[:],
                    pt_[:, c4 * 512 : (c4 + 1) * 512],
                    start=(t == 0),
                    stop=(t == NT - 1),
                )

        # ---- count: 4 accumulation chains on the software DGE, issued
        # chain-interleaved so the in-order gpsimd engine keeps 4 DMAs in
        # flight; then a DMA merge tree and one ACT fold with accum_out.
        NCH = 4
        acc8s = []
        for c in range(NCH):
            acc8_c = const.tile([P, N], u8, tag=f"acc8_{c}")
            acc8s.append(acc8_c)
        for step in range(NT // NCH):
            for c in range(NCH):
                nc.gpsimd.dma_start(
                    acc8s[c][:],
                    mts[c * (NT // NCH) + step][:],
                    accum_op=(OP.bypass if step == 0 else OP.add),
                )
        nc.gpsimd.dma_start(acc8s[0][:], acc8s[1][:], accum_op=OP.add)
        nc.gpsimd.dma_start(acc8s[2][:], acc8s[3][:], accum_op=OP.add)
        nc.gpsimd.dma_start(acc8s[0][:], acc8s[2][:], accum_op=OP.add)
        cnt_col = const.tile([P, 1], f32)
        csc = const.tile([P, N], f16)
        nc.scalar.activation(csc[:], acc8s[0][:], AF.Copy, accum_out=cnt_col[:])
        nc.vector.tensor_copy(partials[:, 3:4], cnt_col[:])

        # masked diagonal count -> partials[:, 2] (sync queue is idle now)
        diag_u8 = const.tile([P, NT], u8)
        diag_ap = mask_d.ap().rearrange("i j -> (i j)")[:: N + 1].rearrange(
            "(p t) -> p t", t=NT
        )
        nc.sync.dma_start(diag_u8[:], diag_ap)
        diag_f = const.tile([P, NT], f32)
        nc.vector.tensor_copy(diag_f[:], diag_u8[:])
        nc.vector.tensor_reduce(
            out=partials[:, 2:3], in_=diag_f[:], axis=AX.X, op=OP.add
        )

        # ---- pull tail: softplus(x) = ln(1 + exp(x)) uses ACT set 6
        # (exp+ln), loaded once here after all set-2 work is done
        sp_e = const.tile([NT, P], f32)
        nc.scalar.activation(sp_e[:], x_sb[:], AF.Exp)
        sp_e1 = const.tile([NT, P], f32)
        nc.vector.tensor_scalar(
            out=sp_e1[:], in0=sp_e[:], scalar1=1.0, scalar2=None, op0=OP.add
        )
        sp = const.tile([NT, P], f32)
        nc.scalar.activation(sp[:], sp_e1[:], AF.Ln)
        tag = const.tile([NT, P], f32)
        nc.vector.tensor_tensor(out=tag[:], in0=sp[:], in1=xt[:], op=OP.subtract)
        wt = const.tile([NT, P], f32)
        nc.vector.tensor_tensor(out=wt[:], in0=tag[:], in1=c_sb[:], op=OP.mult)
        nc.vector.tensor_reduce(
            out=partials[0:NT, 0:1], in_=wt[:], axis=AX.X, op=OP.add
        )

        # ---------------- final reductions ----------------
        nc.vector.tensor_reduce(
            out=partials[0:1, 4:5], in_=psum_abs[:], axis=AX.X, op=OP.add
        )
        # transpose partials via DRAM bounce, reduce to [8, 1]
        nc.sync.dma_start(part_d.ap(), partials[:])
        pt8 = const.tile([N_PARTIALS, P], f32)
        nc.sync.dma_start(pt8[:], part_d.ap().rearrange("p c -> c p"))
        out_sb = const.tile([N_PARTIALS, 1], f32)
        nc.vector.tensor_reduce(out=out_sb[:], in_=pt8[:], axis=AX.X, op=OP.add)
        nc.sync.dma_start(out_d.ap(), out_sb[:])

    nc.compile()
    return nc


_NC_CACHE = None


def _get_nc():
    global _NC_CACHE
    if _NC_CACHE is None:
        _NC_CACHE = build_kernel()
    return _NC_CACHE


def _make_in_maps(
    lof_tag_img, lof_tag_avg_img, lof_tag_avg_gather_img, mask, centerness_img
):
    in_maps = []
    for k in range(N_CORES):
        in_maps.append(
            {
                "mask": np.ascontiguousarray(mask[k]).view(np.uint8),
                "avg": np.ascontiguousarray(
                    lof_tag_avg_img[k], dtype=np.float32
                ).reshape(NT, P),
                "x": np.ascontiguousarray(
                    lof_tag_img[k], dtype=np.float32
                ).reshape(NT, P),
                "g": np.ascontiguousarray(
                    lof_tag_avg_gather_img[k], dtype=np.float32
                ).reshape(NT, P),
                "cen": np.ascontiguousarray(
                    centerness_img[k], dtype=np.float32
                ).reshape(NT, P),
            }
        )
    return in_maps


def _dup_column_correction(avg, mask):
    """count correction for duplicate sigmoid columns (all-batch-equal
    pairs beyond the diagonal). Zero for generic random inputs."""
    s = (1.0 / (1.0 + np.exp(-avg.astype(np.float32)))).astype(np.float32)
    cols = np.ascontiguousarray(s.T)  # [N, B]
    _, inv, counts = np.unique(
        cols.view([("", cols.dtype)] * cols.shape[1]).ravel(),
        return_inverse=True,
        return_counts=True,
    )
    corr = 0.0
    if np.any(counts > 1):
        for gid in np.nonzero(counts > 1)[0]:
            idx = np.nonzero(inv == gid)[0]
            for i in idx:
                for j in idx:
                    if i != j:
                        corr += float(mask[:, i, j].sum())
    return corr


def _combine(partials_per_core, avg, mask):
    tot = np.sum(
        [p.reshape(-1).astype(np.float64) for p in partials_per_core], axis=0
    )
    pull_num, pull_den, diag_cnt, cnt_raw, abssum = tot[:5]
    pull = pull_num / pull_den
    count = cnt_raw - diag_cnt - _dup_column_correction(avg, mask)
    if count > 0:
        push = (THR * count - abssum) / count
    else:
        push = 0.0
    return np.float32(pull), np.float32(push)


def kernel(lof_tag_img, lof_tag_avg_img, lof_tag_avg_gather_img, mask, centerness_img):
    from concourse import bass_utils

    nc = _get_nc()
    in_maps = _make_in_maps(
        lof_tag_img, lof_tag_avg_img, lof_tag_avg_gather_img, mask, centerness_img
    )
    res = bass_utils.run_bass_kernel_spmd(
        nc, in_maps, core_ids=list(range(N_CORES))
    )
    partials = [res.results[k]["out"] for k in range(N_CORES)]
    return _combine(
        partials, np.asarray(lof_tag_avg_img), np.asarray(mask)
    )


# revision 25
# speedup vs baseline: 1.3252x; 1.3252x over previous
"""Trainium2 Bass kernel for nn_AELossV2 (loss_fn).

Full inputs -> (pull, push) scalars.

Strategy: data-parallel over batch B=8 across 8 NeuronCores. Core k
processes mask[k] ([2048, 2048] u8, the only large tensor) plus its
[2048]-row slices of the small tensors, producing 5 scalar partials:
  [pull_num, pull_den, diag_masked_cnt, mask_cnt_raw, abssum]
The host sums partials over cores and forms the two output scalars.

Per-core math (s = sigmoid(avg_row), thr = 0.6):
  abssum = sum_ij mask[i,j] * |s[j] - s[i]|     (dist_mask is implicit:
           pairs excluded by dist_mask have s equal in EVERY batch row,
           so their |s[j]-s[i]| term is 0 in every core's sum already)
  count  = sum_ij mask[i,j] - sum_i mask[i,i]   (- host-side correction
           for duplicate s-columns, which never occur for random data)
  push   = (thr*count - abssum) / count
  pull   = sum(tag * c) / sum(c),  tag = softplus(x) - x * (g > 0)

Engine split per [128, 2048] tile of the [N, N] plane:
  ACT:  at = Abs(s_brd_psum - s_col)   (|d|; reads s_brd from PSUM)
  DVE:  P  = min(mask_u8, at)          (masking without a cast: u8 1
        converts to 1.0 > |d|, so min selects |d|; 0 selects 0)
  PE :  column sums of P accumulated into PSUM  (-> abssum)
  DMA:  software-DGE accum-DMAs fold the mask tiles into u8
        accumulators (values <= 16) -> count costs ~no engine time.
s_brd is built by a PE rank-1 broadcast (block-identity @ s_row) and
stays in PSUM; s appears in both layouts via two ACT Sigmoid calls on
two differently-laid-out DMA copies of avg, so no transpose is needed.
"""

import sys
from contextlib import ExitStack

import numpy as np

try:
    import concourse.bass  # noqa: F401
except ImportError:  # pragma: no cover
    sys.path.insert(0, "/opt/trn_rl_repo")

B = 8
N = 2048
P = 128
NT = N // P  # 16 tiles per plane
THR = 0.5 + 0.1
N_CORES = 8
N_PARTIALS = 8  # padded partials vector


def build_kernel():
    import concourse.bass as bass
    import concourse.tile as tile
    from concourse import bacc, mybir

    f16 = mybir.dt.float16
    f32 = mybir.dt.float32
    u8 = mybir.dt.uint8
    AF = mybir.ActivationFunctionType
    OP = mybir.AluOpType
    AX = mybir.AxisListType

    nc = bacc.Bacc("TRN2", target_bir_lowering=False, debug=False)

    mask_d = nc.dram_tensor("mask", [N, N], u8, kind="ExternalInput")
    avg_d = nc.dram_tensor("avg", [NT, P], f32, kind="ExternalInput")
    x_d = nc.dram_tensor("x", [NT, P], f32, kind="ExternalInput")
    g_d = nc.dram_tensor("g", [NT, P], f32, kind="ExternalInput")
    c_d = nc.dram_tensor("cen", [NT, P], f32, kind="ExternalInput")
    out_d = nc.dram_tensor("out", [N_PARTIALS, 1], f32, kind="ExternalOutput")

    part_d = nc.dram_tensor("part_scratch", [P, N_PARTIALS], f32)

    with tile.TileContext(nc) as tc, ExitStack() as ctx:
        const = ctx.enter_context(tc.tile_pool(name="const", bufs=1))
        mpool = ctx.enter_context(tc.tile_pool(name="masku8", bufs=1))
        apool = ctx.enter_context(tc.tile_pool(name="absd", bufs=6))
        ppool = ctx.enter_context(tc.tile_pool(name="prod", bufs=6))
        pspool = ctx.enter_context(
            tc.tile_pool(name="ps", bufs=1, space=bass.MemorySpace.PSUM)
        )

        # ---- DMA plan: two tiny avg loads first, then the mask tiles
        # split across the two HWDGE queues (sync + scalar). Small pull
        # inputs ride the software DGE so they never queue behind masks.
        avg_pt = const.tile([P, NT], f32)
        nc.sync.dma_start(avg_pt[:], avg_d.ap().rearrange("t p -> p t"))
        avg_row = const.tile([NT, P], f32)
        nc.sync.dma_start(avg_row[:], avg_d.ap())
        diag_u8 = const.tile([P, NT], u8)
        diag_ap = mask_d.ap().rearrange("i j -> (i j)")[:: N + 1].rearrange(
            "(p t) -> p t", t=NT
        )
        nc.sync.dma_start(diag_u8[:], diag_ap)
        mts = []
        for t in range(NT):
            mt = mpool.tile([P, N], u8, tag=f"mt{t}")
            eng = nc.sync if t % 2 == 0 else nc.scalar
            eng.dma_start(mt[:], mask_d.ap()[t * P : (t + 1) * P, :])
            mts.append(mt)
        x_sb = const.tile([NT, P], f32)
        g_sb = const.tile([NT, P], f32)
        c_sb = const.tile([NT, P], f32)
        nc.gpsimd.dma_start(x_sb[:], x_d.ap())
        nc.gpsimd.dma_start(g_sb[:], g_d.ap())
        nc.gpsimd.dma_start(c_sb[:], c_d.ap())

        # ---- s in both layouts via sigmoid = 1/(1+exp(-x)): keeps every
        # ACT func (Exp, Ln, Abs, Copy) inside one table set -> one load
        sc_e = const.tile([P, NT], f32)
        nc.scalar.activation(sc_e[:], avg_pt[:], AF.Exp, scale=-1.0)
        sc_e1 = const.tile([P, NT], f32)
        nc.vector.tensor_scalar(
            out=sc_e1[:], in0=sc_e[:], scalar1=1.0, scalar2=None, op0=OP.add
        )
        sc_f32 = const.tile([P, NT], f32)
        nc.vector.reciprocal(sc_f32[:], sc_e1[:])
        sr_e = const.tile([NT, P], f32)
        nc.scalar.activation(sr_e[:], avg_row[:], AF.Exp, scale=-1.0)
        sr_e1 = const.tile([NT, P], f32)
        nc.vector.tensor_scalar(
            out=sr_e1[:], in0=sr_e[:], scalar1=1.0, scalar2=None, op0=OP.add
        )
        sr_f32 = const.tile([NT, P], f32)
        nc.vector.reciprocal(sr_f32[:], sr_e1[:])
        s16c = const.tile([P, NT], f16)
        nc.vector.tensor_copy(s16c[:], sc_f32[:])
        sneg_col = const.tile([P, NT], f32)
        nc.vector.tensor_scalar(
            out=sneg_col[:], in0=s16c[:], scalar1=-1.0, scalar2=None, op0=OP.mult
        )
        s16row = const.tile([NT, P], f16)
        nc.vector.tensor_copy(s16row[:], sr_f32[:])

        # ---- s_brd: PE rank-1 broadcast into PSUM (stays there; the
        # per-tile Abs reads PSUM directly). lhsT is a block-identity
        # selector column broadcast along the free axis.
        iota_t = const.tile([NT, NT], mybir.dt.int16)
        nc.gpsimd.iota(iota_t[:], pattern=[[1, NT]], channel_multiplier=-1)
        id16 = const.tile([NT, NT], f16)
        nc.vector.tensor_scalar(
            out=id16[:], in0=iota_t[:], scalar1=0.0, scalar2=None, op0=OP.is_equal
        )
        psum_brd = pspool.tile([P, N], f32)
        for t in range(NT):
            nc.tensor.matmul(
                psum_brd[:, t * P : (t + 1) * P],
                id16[:, t : t + 1].broadcast_to([NT, P]),
                s16row[:, :],
                start=True,
                stop=True,
            )

        partials = const.tile([P, N_PARTIALS], f32)
        nc.vector.memset(partials[:], 0.0)
        ones = const.tile([P, 1], f16)
        nc.vector.memset(ones[:], 1.0)

        # pull pieces that need no ACT (its tables are busy with set 2)
        tgt = const.tile([NT, P], f32)
        nc.vector.tensor_scalar(
            out=tgt[:], in0=g_sb[:], scalar1=0.0, scalar2=None, op0=OP.is_gt
        )
        xt = const.tile([NT, P], f32)
        nc.vector.tensor_tensor(out=xt[:], in0=x_sb[:], in1=tgt[:], op=OP.mult)
        nc.vector.tensor_reduce(
            out=partials[0:NT, 1:2], in_=c_sb[:], axis=AX.X, op=OP.add
        )

        # ---------------- the [N, N] plane loop ----------------
        psum_abs = pspool.tile([1, N], f32)
        for t in range(NT):
            mt = mts[t]
            # |s_j - s_i| on ACT: Abs(1.0 * s_brd + (-s_col)), PSUM src
            at = apool.tile([P, N], f16)
            nc.scalar.activation(
                at[:], psum_brd[:, :], AF.Abs, bias=sneg_col[:, t : t + 1]
            )
            # masked |d| on DVE: mask==1 -> 1.0 > |d|, min picks |d|
            pt_ = ppool.tile([P, N], f16)
            nc.vector.tensor_tensor(out=pt_[:], in0=mt[:], in1=at[:], op=OP.min)
            if t == NT - 1:
                last_pt = pt_
            # column sums into the PSUM accumulator
            for c4 in range(4):
                nc.tensor.matmul(
                    psum_abs[0:1, c4 * 512 : (c4 + 1) * 512],
                    ones[:],
                    pt_[:, c4 * 512 : (c4 + 1) * 512],
                    start=(t == 0),
                    stop=(t == NT - 1),
                )

        # ---- count: 4 accumulation chains on the software DGE, issued
        # chain-interleaved so the in-order gpsimd engine keeps 4 DMAs in
        # flight; then a DMA merge tree and one ACT fold with accum_out.
        NCH = 4
        acc8s = []
        for c in range(NCH):
            acc8_c = const.tile([P, N], u8, tag=f"acc8_{c}")
            acc8s.append(acc8_c)
        for step in range(NT // NCH):
            for c in range(NCH):
                nc.gpsimd.dma_start(
                    acc8s[c][:],
                    mts[step * NCH + c][:],
                    accum_op=(OP.bypass if step == 0 else OP.add),
                )
        nc.gpsimd.dma_start(acc8s[0][:], acc8s[1][:], accum_op=OP.add)
        nc.gpsimd.dma_start(acc8s[2][:], acc8s[3][:], accum_op=OP.add)
        nc.gpsimd.dma_start(acc8s[0][:], acc8s[2][:], accum_op=OP.add)
        cnt_col = const.tile([P, 1], f32)
        # zero-seed written from the last product tile: a WAW dependency
        # that forces the fold to be scheduled after the plane finishes
        nc.vector.tensor_scalar(
            out=cnt_col[:], in0=last_pt[:, 0:1], scalar1=0.0, scalar2=None,
            op0=OP.mult,
        )
        csc = const.tile([P, N], f16)
        nc.scalar.activation(csc[:], acc8s[0][:], AF.Copy, accum_out=cnt_col[:])
        nc.vector.tensor_copy(partials[:, 3:4], cnt_col[:])

        # masked diagonal count -> partials[:, 2]
        diag_f = const.tile([P, NT], f32)
        nc.vector.tensor_copy(diag_f[:], diag_u8[:])
        nc.vector.tensor_reduce(
            out=partials[:, 2:3], in_=diag_f[:], axis=AX.X, op=OP.add
        )

        # ---- pull tail: softplus(x) = ln(1 + exp(x)) uses ACT set 6
        # (exp+ln), loaded once here after all set-2 work is done
        sp_e = const.tile([NT, P], f32)
        nc.scalar.activation(sp_e[:], x_sb[:], AF.Exp)
        sp_e1 = const.tile([NT, P], f32)
        nc.vector.tensor_scalar(
            out=sp_e1[:], in0=sp_e[:], scalar1=1.0, scalar2=None, op0=OP.add
        )
        sp = const.tile([NT, P], f32)
        nc.scalar.activation(sp[:], sp_e1[:], AF.Ln)
        tag = const.tile([NT, P], f32)
        nc.vector.tensor_tensor(out=tag[:], in0=sp[:], in1=xt[:], op=OP.subtract)
        wt = const.tile([NT, P], f32)
        nc.vector.tensor_tensor(out=wt[:], in0=tag[:], in1=c_sb[:], op=OP.mult)
        nc.vector.tensor_reduce(
            out=partials[0:NT, 0:1], in_=wt[:], axis=AX.X, op=OP.add
        )

        # ---------------- final reductions ----------------
        nc.vector.tensor_reduce(
            out=partials[0:1, 4:5], in_=psum_abs[:], axis=AX.X, op=OP.add
        )
        # transpose partials via DRAM bounce, reduce to [8, 1]
        nc.sync.dma_start(part_d.ap(), partials[:])
        pt8 = const.tile([N_PARTIALS, P], f32)
        nc.sync.dma_start(pt8[:], part_d.ap().rearrange("p c -> c p"))
        out_sb = const.tile([N_PARTIALS, 1], f32)
        nc.vector.tensor_reduce(out=out_sb[:], in_=pt8[:], axis=AX.X, op=OP.add)
        nc.sync.dma_start(out_d.ap(), out_sb[:])

    nc.compile()
    return nc


_NC_CACHE = None


def _get_nc():
    global _NC_CACHE
    if _NC_CACHE is None:
        _NC_CACHE = build_kernel()
    return _NC_CACHE


def _make_in_maps(
    lof_tag_img, lof_tag_avg_img, lof_tag_avg_gather_img, mask, centerness_img
):
    in_maps = []
    for k in range(N_CORES):
        in_maps.append(
            {
                "mask": np.ascontiguousarray(mask[k]).view(np.uint8),
                "avg": np.ascontiguousarray(
                    lof_tag_avg_img[k], dtype=np.float32
                ).reshape(NT, P),
                "x": np.ascontiguousarray(
                    lof_tag_img[k], dtype=np.float32
                ).reshape(NT, P),
                "g": np.ascontiguousarray(
                    lof_tag_avg_gather_img[k], dtype=np.float32
                ).reshape(NT, P),
                "cen": np.ascontiguousarray(
                    centerness_img[k], dtype=np.float32
                ).reshape(NT, P),
            }
        )
    return in_maps


def _dup_column_correction(avg, mask):
    """count correction for duplicate sigmoid columns (all-batch-equal
    pairs beyond the diagonal). Zero for generic random inputs."""
    s = (1.0 / (1.0 + np.exp(-avg.astype(np.float32)))).astype(np.float32)
    cols = np.ascontiguousarray(s.T)  # [N, B]
    _, inv, counts = np.unique(
        cols.view([("", cols.dtype)] * cols.shape[1]).ravel(),
        return_inverse=True,
        return_counts=True,
    )
    corr = 0.0
    if np.any(counts > 1):
        for gid in np.nonzero(counts > 1)[0]:
            idx = np.nonzero(inv == gid)[0]
            for i in idx:
                for j in idx:
                    if i != j:
                        corr += float(mask[:, i, j].sum())
    return corr


def _combine(partials_per_core, avg, mask):
    tot = np.sum(
        [p.reshape(-1).astype(np.float64) for p in partials_per_core], axis=0
    )
    pull_num, pull_den, diag_cnt, cnt_raw, abssum = tot[:5]
    pull = pull_num / pull_den
    count = cnt_raw - diag_cnt - _dup_column_correction(avg, mask)
    if count > 0:
        push = (THR * count - abssum) / count
    else:
        push = 0.0
    return np.float32(pull), np.float32(push)


def kernel(lof_tag_img, lof_tag_avg_img, lof_tag_avg_gather_img, mask, centerness_img):
    from concourse import bass_utils

    nc = _get_nc()
    in_maps = _make_in_maps(
        lof_tag_img, lof_tag_avg_img, lof_tag_avg_gather_img, mask, centerness_img
    )
    res = bass_utils.run_bass_kernel_spmd(
        nc, in_maps, core_ids=list(range(N_CORES))
    )
    partials = [res.results[k]["out"] for k in range(N_CORES)]
    return _combine(
        partials, np.asarray(lof_tag_avg_img), np.asarray(mask)
    )
